# revision 15
# baseline (speedup 1.0000x reference)
"""MoE routing kernel for Trainium2 (8 NeuronCores, expert-parallel).

Problem (hardcoded): B=1024 samples, each with a 14x14 mask (flattened to
D=196 features), routed by `instance[b]` to one of E=16 two-layer MLP
experts: Linear(196,512) -> ReLU -> Linear(512,1024).  Output [1024,1024] f32.

Strategy: on host, group samples by expert into chunks of <=128 samples.
With random routing there are exactly 16 chunks (one per expert), i.e. 2
chunks ("slots") per core across 8 cores.  Each core runs its slots'
expert MLPs on its gathered samples; the host scatters rows back.

Device kernel (per slot):
  hT[H,C] = relu(W1^T[H,D] @ xT[D,C] + b1)   (H on psum partitions -> hT lands
                                              already transposed for layer 2)
  y[C,A]  = hT^T @ W2 + b2                   (C on psum partitions)
Contraction dims are padded/tiled to 128-partition chunks: D 196->256 (2
chunks), H=512 (4 chunks).  Inputs are packed host-side into two
partition-major blobs per slot (a: xT|W1, b: W2) so each slot needs just two
large DMAs that fan out across all DMA queues; ReLU runs on the Vector engine
(no ACT table load) and y returns as bf16 (f32 for the f32 build).
b2 (if nonzero) is seeded into PSUM with a rank-1 matmul.  The build is
specialized at compile time to whether b1/b2 are actually nonzero.
"""

import numpy as np

import concourse.bacc as bacc
import concourse.mybir as mybir
import concourse.tile as tile
from concourse.bass import ts
from concourse.bass_utils import run_bass_kernel_spmd

E = 16
D = 196
DP = 256
H = 512
A = 1024
B = 1024
P = 128
NCORES = 8
SLOTS = 2
KD = DP // P
KH = H // P
NF = 512          # matmul free-dim tile for layer 2 output
NA = A // NF
FA = KD * P + KD * H   # 1280 per-partition elements: [xT | W1]
FB = KH * A            # 4096 per-partition elements: [W2]

# COMPUTE_DT options: "f32", "f32r", "bf16"
COMPUTE_DT = "bf16"

_NC_CACHE = {}
LAST_RESULTS = None


def _dtypes(compute_dt):
    if compute_dt == "bf16":
        import ml_dtypes

        return mybir.dt.bfloat16, ml_dtypes.bfloat16
    if compute_dt == "f32r":
        return mybir.dt.float32r, np.float32
    return mybir.dt.float32, np.float32


def _build(compute_dt, with_b1, with_b2):
    cdt, _ = _dtypes(compute_dt)
    f32 = mybir.dt.float32
    y_dt = mybir.dt.bfloat16 if compute_dt == "bf16" else f32
    nc = bacc.Bacc("TRN2", target_bir_lowering=False)

    a_d = nc.dram_tensor("a", [SLOTS, P, FA], cdt, kind="ExternalInput")
    b_d = nc.dram_tensor("b", [SLOTS, P, FB], cdt, kind="ExternalInput")
    b1_d = (
        nc.dram_tensor("b1", [SLOTS, P, KH], f32, kind="ExternalInput")
        if with_b1
        else None
    )
    b2_d = (
        nc.dram_tensor("b2", [SLOTS, A], cdt, kind="ExternalInput")
        if with_b2
        else None
    )
    y_d = nc.dram_tensor("y", [SLOTS, P, A], y_dt, kind="ExternalOutput")

    with tile.TileContext(nc) as tc:
        with (
            tc.tile_pool(name="const", bufs=1) as const,
            tc.tile_pool(name="sb", bufs=2) as sb,
            tc.tile_pool(name="ps", bufs=2, space="PSUM") as ps,
        ):
            # Issue every input DMA up front, split across two issue engines
            # so descriptor generation parallelizes and all queues stream.
            # W2 goes as per-m-chunk DMAs in exact consumption order so each
            # mm2 accumulation step starts as soon as its chunk lands.
            a_ts = []
            b_ts = [[None] * KH for _ in range(SLOTS)]
            b1_ts = []
            for s in range(SLOTS):
                a_t = sb.tile([P, FA], cdt, tag="a")
                nc.sync.dma_start(a_t[:], a_d[s])
                a_ts.append(a_t)
            # W2 chunks in exact consumption order (s/m interleaved) on the
            # scalar ring.
            for m in range(KH):
                for s in range(SLOTS):
                    b_t = sb.tile([P, A], cdt, tag="w2m", bufs=2 * KH)
                    nc.scalar.dma_start(b_t[:], b_d[s][:, ts(m, A)])
                    b_ts[s][m] = b_t
            if with_b1:
                for s in range(SLOTS):
                    b1_t = sb.tile([P, KH], f32, tag="b1")
                    nc.sync.dma_start(b1_t[:], b1_d[s])
                    b1_ts.append(b1_t)
            if with_b2:
                e0 = const.tile([P, P], cdt, tag="e0")
                nc.any.memset(e0[:], 0.0)
                nc.any.memset(e0[0:1, :], 1.0)
                b2_ts = []
                for s in range(SLOTS):
                    b2_t = const.tile([P, A], cdt, tag=f"b2_{s}")
                    nc.any.memset(b2_t[:], 0.0)
                    nc.sync.dma_start(b2_t[0:1, :], b2_d[s][None, :])
                    b2_ts.append(b2_t)

            hTs = []
            y_ts = []
            p2s = []
            for s in range(SLOTS):
                xt_v = a_ts[s][:, : KD * P].rearrange("p (o c) -> p o c", o=KD)
                w1_v = a_ts[s][:, KD * P :].rearrange("p (o h) -> p o h", o=KD)

                hT = sb.tile([P, KH, P], cdt, tag="hT")
                for m in range(KH):
                    p1 = ps.tile([P, P], f32, tag="p1")
                    for o in range(KD):
                        nc.tensor.matmul(
                            p1[:],
                            w1_v[:, o, ts(m, P)],
                            xt_v[:, o, :],
                            start=(o == 0),
                            stop=(o == KD - 1),
                        )
                    if with_b1:
                        nc.vector.tensor_scalar(
                            hT[:, m, :],
                            p1[:],
                            b1_ts[s][:, m : m + 1],
                            0.0,
                            mybir.AluOpType.add,
                            mybir.AluOpType.max,
                        )
                    else:
                        nc.vector.tensor_scalar_max(hT[:, m, :], p1[:], 0.0)
                hTs.append(hT)
                y_ts.append(sb.tile([P, A], y_dt, tag="y", name=f"y_{s}"))
                p2s.append(ps.tile([P, NA, NF], f32, tag="p2", name=f"p2_{s}"))

            # Layer 2, interleaved across slots in W2-chunk arrival order.
            for m in range(KH):
                for s in range(SLOTS):
                    if with_b2 and m == 0:
                        for n in range(NA):
                            nc.tensor.matmul(
                                p2s[s][:, n, :],
                                e0[:],
                                b2_ts[s][:, ts(n, NF)],
                                start=True,
                                stop=False,
                            )
                    for n in range(NA):
                        nc.tensor.matmul(
                            p2s[s][:, n, :],
                            hTs[s][:, m, :],
                            b_ts[s][m][:, ts(n, NF)],
                            start=(m == 0 and not with_b2),
                            stop=(m == KH - 1),
                        )
                        if m == KH - 1:
                            nc.vector.tensor_copy(
                                y_ts[s][:, ts(n, NF)], p2s[s][:, n, :]
                            )
                            nc.sync.dma_start(
                                y_d[s][:, ts(n, NF)], y_ts[s][:, ts(n, NF)]
                            )

    nc.compile()
    return nc


def _get_nc(compute_dt, with_b1, with_b2):
    key = (compute_dt, with_b1, with_b2)
    if key not in _NC_CACHE:
        _NC_CACHE[key] = _build(*key)
    return _NC_CACHE[key]


def kernel(**inputs):
    global LAST_RESULTS
    mask = np.ascontiguousarray(np.asarray(inputs["mask"], dtype=np.float32))
    instance = np.asarray(inputs["instance"]).astype(np.int64)
    W1 = np.asarray(inputs["W1"], dtype=np.float32)
    b1 = np.asarray(inputs["b1"], dtype=np.float32)
    W2 = np.asarray(inputs["W2"], dtype=np.float32)
    b2 = np.asarray(inputs["b2"], dtype=np.float32)

    cdt, npdt = _dtypes(COMPUTE_DT)
    with_b1 = bool(np.any(b1))
    with_b2 = bool(np.any(b2))
    nc = _get_nc(COMPUTE_DT, with_b1, with_b2)

    x = mask.reshape(B, D)
    xp = np.zeros((B, DP), np.float32)
    xp[:, :D] = x
    xp = xp.astype(npdt, copy=False)

    # Weight layouts matching the SBUF tiles: partition dim first.
    W1p = np.zeros((E, DP, H), np.float32)
    W1p[:, :D, :] = W1
    w1_l = np.ascontiguousarray(
        W1p.reshape(E, KD, P, H).transpose(0, 2, 1, 3).reshape(E, P, KD * H)
    ).astype(npdt, copy=False)                            # [E, P, KD*H]
    w2_l = np.ascontiguousarray(
        W2.reshape(E, KH, P, A).transpose(0, 2, 1, 3).reshape(E, P, FB)
    ).astype(npdt, copy=False)                            # [E, P, FB]
    b1_l = np.ascontiguousarray(b1.reshape(E, KH, P).transpose(0, 2, 1))  # [E,P,KH]
    b2_l = b2.astype(npdt, copy=False)

    chunks = []
    for e in range(E):
        idx = np.nonzero(instance == e)[0]
        for i in range(0, len(idx), P):
            chunks.append((e, idx[i : i + P]))
    per_round = NCORES * SLOTS
    rounds = max(1, -(-len(chunks) // per_round))

    y = np.zeros((B, A), np.float32)
    for r in range(rounds):
        in_maps = []
        slot_idx = []  # (core, slot) -> sample indices
        for c in range(NCORES):
            ab = np.zeros((SLOTS, P, FA), npdt)
            bb = np.zeros((SLOTS, P, FB), npdt)
            b1a = np.zeros((SLOTS, P, KH), np.float32)
            b2a = np.zeros((SLOTS, A), npdt)
            cidx = []
            for s in range(SLOTS):
                k = r * per_round + c * SLOTS + s
                if k < len(chunks):
                    e, idx = chunks[k]
                    L = len(idx)
                    xg = xp[idx]  # [L, DP]
                    xt = ab[s, :, : KD * P].reshape(P, KD, P)
                    for o in range(KD):
                        xt[:, o, :L] = xg[:, o * P : (o + 1) * P].T
                    ab[s, :, KD * P :] = w1_l[e]
                    bb[s] = w2_l[e]
                    b1a[s] = b1_l[e]
                    b2a[s] = b2_l[e]
                    cidx.append(idx)
                else:
                    cidx.append(None)
            slot_idx.append(cidx)
            m = {"a": ab, "b": bb}
            if with_b1:
                m["b1"] = b1a
            if with_b2:
                m["b2"] = b2a
            in_maps.append(m)

        res = run_bass_kernel_spmd(nc, in_maps, core_ids=list(range(NCORES)))
        LAST_RESULTS = res
        for c in range(NCORES):
            yc = np.asarray(res.results[c]["y"], dtype=np.float32)
            for s in range(SLOTS):
                idx = slot_idx[c][s]
                if idx is not None:
                    y[idx] = yc[s, : len(idx)]

    return y


# revision 17
# speedup vs baseline: 1.0820x; 1.0820x over previous
"""MoE routing kernel for Trainium2 (8 NeuronCores, expert-parallel).

Problem (hardcoded): B=1024 samples, each with a 14x14 mask (flattened to
D=196 features), routed by `instance[b]` to one of E=16 two-layer MLP
experts: Linear(196,512) -> ReLU -> Linear(512,1024).  Output [1024,1024] f32.

Strategy: on host, group samples by expert into chunks of <=128 samples.
With random routing there are exactly 16 chunks (one per expert), i.e. 2
chunks ("slots") per core across 8 cores.  Each core runs its slots'
expert MLPs on its gathered samples; the host scatters rows back.

Device kernel (per slot):
  hT[H,C] = relu(W1^T[H,D] @ xT[D,C] + b1)   (H on psum partitions -> hT lands
                                              already transposed for layer 2)
  y[C,A]  = hT^T @ W2 + b2                   (C on psum partitions)
Contraction dims are padded/tiled to 128-partition chunks: D 196->256 (2
chunks), H=512 (4 chunks).  Inputs are packed host-side into two
partition-major blobs per slot (a: xT|W1, b: W2) so each slot needs just two
large DMAs that fan out across all DMA queues; ReLU runs on the Vector engine
(no ACT table load) and y returns as bf16 (f32 for the f32 build).
b2 (if nonzero) is seeded into PSUM with a rank-1 matmul.  The build is
specialized at compile time to whether b1/b2 are actually nonzero.
"""

import numpy as np

import concourse.bacc as bacc
import concourse.mybir as mybir
import concourse.tile as tile
from concourse.bass import ts
from concourse.bass_utils import run_bass_kernel_spmd

E = 16
D = 196
DP = 256
H = 512
A = 1024
B = 1024
P = 128
NCORES = 8
SLOTS = 2
KD = DP // P
KH = H // P
NF = 512          # matmul free-dim tile for layer 2 output
NA = A // NF
FA = KD * P + KD * H   # 1280 per-partition elements: [xT | W1]
FB = KH * A            # 4096 per-partition elements: [W2]

# COMPUTE_DT options: "f32", "f32r", "bf16"
COMPUTE_DT = "bf16"

_NC_CACHE = {}
LAST_RESULTS = None


def _dtypes(compute_dt):
    if compute_dt == "bf16":
        import ml_dtypes

        return mybir.dt.bfloat16, ml_dtypes.bfloat16
    if compute_dt == "f32r":
        return mybir.dt.float32r, np.float32
    return mybir.dt.float32, np.float32


def _build(compute_dt, with_b1, with_b2):
    cdt, _ = _dtypes(compute_dt)
    f32 = mybir.dt.float32
    y_dt = mybir.dt.bfloat16 if compute_dt == "bf16" else f32
    nc = bacc.Bacc("TRN2", target_bir_lowering=False)

    a_d = nc.dram_tensor("a", [SLOTS, P, FA], cdt, kind="ExternalInput")
    b_d = nc.dram_tensor("b", [SLOTS, P, FB], cdt, kind="ExternalInput")
    b1_d = (
        nc.dram_tensor("b1", [SLOTS, P, KH], f32, kind="ExternalInput")
        if with_b1
        else None
    )
    b2_d = (
        nc.dram_tensor("b2", [SLOTS, A], cdt, kind="ExternalInput")
        if with_b2
        else None
    )
    y_d = nc.dram_tensor("y", [SLOTS, P, A], y_dt, kind="ExternalOutput")

    with tile.TileContext(nc) as tc:
        with (
            tc.tile_pool(name="const", bufs=1) as const,
            tc.tile_pool(name="sb", bufs=2) as sb,
            tc.tile_pool(name="ps", bufs=2, space="PSUM") as ps,
        ):
            # Issue every input DMA up front, split across two issue engines
            # so descriptor generation parallelizes and all queues stream.
            # W2 goes as per-m-chunk DMAs in exact consumption order so each
            # mm2 accumulation step starts as soon as its chunk lands.
            a_ts = []
            b_ts = [[None] * KH for _ in range(SLOTS)]
            b1_ts = []
            for s in range(SLOTS):
                a_t = sb.tile([P, FA], cdt, tag="a")
                nc.sync.dma_start(a_t[:], a_d[s])
                a_ts.append(a_t)
            # W2 chunks in exact consumption order on the scalar ring:
            # all of slot 0 first so its writeback overlaps slot 1's stream.
            for s in range(SLOTS):
                for m in range(KH):
                    b_t = sb.tile([P, A], cdt, tag="w2m", bufs=2 * KH)
                    nc.scalar.dma_start(b_t[:], b_d[s][:, ts(m, A)])
                    b_ts[s][m] = b_t
            if with_b1:
                for s in range(SLOTS):
                    b1_t = sb.tile([P, KH], f32, tag="b1")
                    nc.sync.dma_start(b1_t[:], b1_d[s])
                    b1_ts.append(b1_t)
            if with_b2:
                e0 = const.tile([P, P], cdt, tag="e0")
                nc.any.memset(e0[:], 0.0)
                nc.any.memset(e0[0:1, :], 1.0)
                b2_ts = []
                for s in range(SLOTS):
                    b2_t = const.tile([P, A], cdt, tag=f"b2_{s}")
                    nc.any.memset(b2_t[:], 0.0)
                    nc.sync.dma_start(b2_t[0:1, :], b2_d[s][None, :])
                    b2_ts.append(b2_t)

            hTs = []
            y_ts = []
            p2s = []
            for s in range(SLOTS):
                xt_v = a_ts[s][:, : KD * P].rearrange("p (o c) -> p o c", o=KD)
                w1_v = a_ts[s][:, KD * P :].rearrange("p (o h) -> p o h", o=KD)

                hT = sb.tile([P, KH, P], cdt, tag="hT")
                for m in range(KH):
                    p1 = ps.tile([P, P], f32, tag="p1")
                    for o in range(KD):
                        nc.tensor.matmul(
                            p1[:],
                            w1_v[:, o, ts(m, P)],
                            xt_v[:, o, :],
                            start=(o == 0),
                            stop=(o == KD - 1),
                        )
                    if with_b1:
                        nc.vector.tensor_scalar(
                            hT[:, m, :],
                            p1[:],
                            b1_ts[s][:, m : m + 1],
                            0.0,
                            mybir.AluOpType.add,
                            mybir.AluOpType.max,
                        )
                    else:
                        nc.vector.tensor_scalar_max(hT[:, m, :], p1[:], 0.0)
                hTs.append(hT)
                y_ts.append(sb.tile([P, A], y_dt, tag="y", name=f"y_{s}"))
                p2s.append(ps.tile([P, NA, NF], f32, tag="p2", name=f"p2_{s}"))

            # Layer 2, in W2-chunk arrival order (slot-major).
            for s in range(SLOTS):
                for m in range(KH):
                    if with_b2 and m == 0:
                        for n in range(NA):
                            nc.tensor.matmul(
                                p2s[s][:, n, :],
                                e0[:],
                                b2_ts[s][:, ts(n, NF)],
                                start=True,
                                stop=False,
                            )
                    for n in range(NA):
                        nc.tensor.matmul(
                            p2s[s][:, n, :],
                            hTs[s][:, m, :],
                            b_ts[s][m][:, ts(n, NF)],
                            start=(m == 0 and not with_b2),
                            stop=(m == KH - 1),
                        )
                        if m == KH - 1:
                            nc.vector.tensor_copy(
                                y_ts[s][:, ts(n, NF)], p2s[s][:, n, :]
                            )
                            nc.sync.dma_start(
                                y_d[s][:, ts(n, NF)], y_ts[s][:, ts(n, NF)]
                            )

    nc.compile()
    return nc


def _get_nc(compute_dt, with_b1, with_b2):
    key = (compute_dt, with_b1, with_b2)
    if key not in _NC_CACHE:
        _NC_CACHE[key] = _build(*key)
    return _NC_CACHE[key]


def kernel(**inputs):
    global LAST_RESULTS
    mask = np.ascontiguousarray(np.asarray(inputs["mask"], dtype=np.float32))
    instance = np.asarray(inputs["instance"]).astype(np.int64)
    W1 = np.asarray(inputs["W1"], dtype=np.float32)
    b1 = np.asarray(inputs["b1"], dtype=np.float32)
    W2 = np.asarray(inputs["W2"], dtype=np.float32)
    b2 = np.asarray(inputs["b2"], dtype=np.float32)

    cdt, npdt = _dtypes(COMPUTE_DT)
    with_b1 = bool(np.any(b1))
    with_b2 = bool(np.any(b2))
    nc = _get_nc(COMPUTE_DT, with_b1, with_b2)

    x = mask.reshape(B, D)
    xp = np.zeros((B, DP), np.float32)
    xp[:, :D] = x
    xp = xp.astype(npdt, copy=False)

    # Weight layouts matching the SBUF tiles: partition dim first.
    W1p = np.zeros((E, DP, H), np.float32)
    W1p[:, :D, :] = W1
    w1_l = np.ascontiguousarray(
        W1p.reshape(E, KD, P, H).transpose(0, 2, 1, 3).reshape(E, P, KD * H)
    ).astype(npdt, copy=False)                            # [E, P, KD*H]
    w2_l = np.ascontiguousarray(
        W2.reshape(E, KH, P, A).transpose(0, 2, 1, 3).reshape(E, P, FB)
    ).astype(npdt, copy=False)                            # [E, P, FB]
    b1_l = np.ascontiguousarray(b1.reshape(E, KH, P).transpose(0, 2, 1))  # [E,P,KH]
    b2_l = b2.astype(npdt, copy=False)

    chunks = []
    for e in range(E):
        idx = np.nonzero(instance == e)[0]
        for i in range(0, len(idx), P):
            chunks.append((e, idx[i : i + P]))
    per_round = NCORES * SLOTS
    rounds = max(1, -(-len(chunks) // per_round))

    y = np.zeros((B, A), np.float32)
    for r in range(rounds):
        in_maps = []
        slot_idx = []  # (core, slot) -> sample indices
        for c in range(NCORES):
            ab = np.zeros((SLOTS, P, FA), npdt)
            bb = np.zeros((SLOTS, P, FB), npdt)
            b1a = np.zeros((SLOTS, P, KH), np.float32)
            b2a = np.zeros((SLOTS, A), npdt)
            cidx = []
            for s in range(SLOTS):
                k = r * per_round + c * SLOTS + s
                if k < len(chunks):
                    e, idx = chunks[k]
                    L = len(idx)
                    xg = xp[idx]  # [L, DP]
                    xt = ab[s, :, : KD * P].reshape(P, KD, P)
                    for o in range(KD):
                        xt[:, o, :L] = xg[:, o * P : (o + 1) * P].T
                    ab[s, :, KD * P :] = w1_l[e]
                    bb[s] = w2_l[e]
                    b1a[s] = b1_l[e]
                    b2a[s] = b2_l[e]
                    cidx.append(idx)
                else:
                    cidx.append(None)
            slot_idx.append(cidx)
            m = {"a": ab, "b": bb}
            if with_b1:
                m["b1"] = b1a
            if with_b2:
                m["b2"] = b2a
            in_maps.append(m)

        res = run_bass_kernel_spmd(nc, in_maps, core_ids=list(range(NCORES)))
        LAST_RESULTS = res
        for c in range(NCORES):
            yc = np.asarray(res.results[c]["y"], dtype=np.float32)
            for s in range(SLOTS):
                idx = slot_idx[c][s]
                if idx is not None:
                    y[idx] = yc[s, : len(idx)]

    return y


# revision 19
# speedup vs baseline: 1.1608x; 1.0729x over previous
"""MoE routing kernel for Trainium2 (8 NeuronCores, expert-parallel).

Problem (hardcoded): B=1024 samples, each with a 14x14 mask (flattened to
D=196 features), routed by `instance[b]` to one of E=16 two-layer MLP
experts: Linear(196,512) -> ReLU -> Linear(512,1024).  Output [1024,1024] f32.

Strategy: on host, group samples by expert into chunks of <=128 samples.
With random routing there are exactly 16 chunks (one per expert), i.e. 2
chunks ("slots") per core across 8 cores.  Each core runs its slots'
expert MLPs on its gathered samples; the host scatters rows back.

Device kernel (per slot):
  hT[H,C] = relu(W1^T[H,D] @ xT[D,C] + b1)   (H on psum partitions -> hT lands
                                              already transposed for layer 2)
  y[C,A]  = hT^T @ W2 + b2                   (C on psum partitions)
Contraction dims are padded/tiled to 128-partition chunks: D 196->256 (2
chunks), H=512 (4 chunks).  Inputs are packed host-side into two
partition-major blobs per slot (a: xT|W1, b: W2) so each slot needs just two
large DMAs that fan out across all DMA queues; ReLU runs on the Vector engine
(no ACT table load) and y returns as bf16 (f32 for the f32 build).
b2 (if nonzero) is seeded into PSUM with a rank-1 matmul.  The build is
specialized at compile time to whether b1/b2 are actually nonzero.
"""

import numpy as np

import concourse.bacc as bacc
import concourse.mybir as mybir
import concourse.tile as tile
from concourse.bass import ts
from concourse.bass_utils import run_bass_kernel_spmd

E = 16
D = 196
DP = 256
H = 512
A = 1024
B = 1024
P = 128
NCORES = 8
SLOTS = 2
KD = DP // P
KH = H // P
NF = 512          # matmul free-dim tile for layer 2 output
NA = A // NF
FA = KD * P + KD * H   # 1280 per-partition elements: [xT | W1]
FB = KH * A            # 4096 per-partition elements: [W2]

# COMPUTE_DT options: "f32", "f32r", "bf16"
COMPUTE_DT = "bf16"

_NC_CACHE = {}
LAST_RESULTS = None


def _dtypes(compute_dt):
    if compute_dt == "bf16":
        import ml_dtypes

        return mybir.dt.bfloat16, ml_dtypes.bfloat16
    if compute_dt == "f32r":
        return mybir.dt.float32r, np.float32
    return mybir.dt.float32, np.float32


def _build(compute_dt, with_b1, with_b2):
    cdt, _ = _dtypes(compute_dt)
    f32 = mybir.dt.float32
    y_dt = mybir.dt.bfloat16 if compute_dt == "bf16" else f32
    nc = bacc.Bacc("TRN2", target_bir_lowering=False)

    a_d = nc.dram_tensor("a", [SLOTS, P, FA], cdt, kind="ExternalInput")
    b_d = nc.dram_tensor("b", [SLOTS, P, FB], cdt, kind="ExternalInput")
    b1_d = (
        nc.dram_tensor("b1", [SLOTS, P, KH], f32, kind="ExternalInput")
        if with_b1
        else None
    )
    b2_d = (
        nc.dram_tensor("b2", [SLOTS, A], cdt, kind="ExternalInput")
        if with_b2
        else None
    )
    y_d = nc.dram_tensor("y", [SLOTS, P, A], y_dt, kind="ExternalOutput")

    with tile.TileContext(nc) as tc:
        with (
            tc.tile_pool(name="const", bufs=1) as const,
            tc.tile_pool(name="sb", bufs=2) as sb,
            tc.tile_pool(name="ps", bufs=2, space="PSUM") as ps,
        ):
            # Issue every input DMA up front, split across two issue engines
            # so descriptor generation parallelizes and all queues stream.
            # W2 goes as per-m-chunk DMAs in exact consumption order so each
            # mm2 accumulation step starts as soon as its chunk lands.
            a_ts = []
            b_ts = [[None] * KH for _ in range(SLOTS)]
            b1_ts = []
            for s in range(SLOTS):
                a_t = sb.tile([P, FA], cdt, tag="a")
                nc.sync.dma_start(a_t[:], a_d[s])
                a_ts.append(a_t)
            # W2 chunks in exact consumption order on the scalar ring:
            # all of slot 0 first so its writeback overlaps slot 1's stream.
            for s in range(SLOTS):
                for m in range(KH):
                    b_t = sb.tile([P, A], cdt, tag="w2m", bufs=2 * KH)
                    nc.scalar.dma_start(b_t[:], b_d[s][:, ts(m, A)])
                    b_ts[s][m] = b_t
            if with_b1:
                for s in range(SLOTS):
                    b1_t = sb.tile([P, KH], f32, tag="b1")
                    nc.sync.dma_start(b1_t[:], b1_d[s])
                    b1_ts.append(b1_t)
            if with_b2:
                e0 = const.tile([P, P], cdt, tag="e0")
                nc.any.memset(e0[:], 0.0)
                nc.any.memset(e0[0:1, :], 1.0)
                b2_ts = []
                for s in range(SLOTS):
                    b2_t = const.tile([P, A], cdt, tag=f"b2_{s}")
                    nc.any.memset(b2_t[:], 0.0)
                    nc.sync.dma_start(b2_t[0:1, :], b2_d[s][None, :])
                    b2_ts.append(b2_t)

            hTs = []
            y_ts = []
            p2s = []
            for s in range(SLOTS):
                xt_v = a_ts[s][:, : KD * P].rearrange("p (o c) -> p o c", o=KD)
                w1_v = a_ts[s][:, KD * P :].rearrange("p (o h) -> p o h", o=KD)

                hT = sb.tile([P, KH, P], cdt, tag="hT")
                for m in range(KH):
                    p1 = ps.tile([P, P], f32, tag="p1")
                    for o in range(KD):
                        nc.tensor.matmul(
                            p1[:],
                            w1_v[:, o, ts(m, P)],
                            xt_v[:, o, :],
                            start=(o == 0),
                            stop=(o == KD - 1),
                        )
                    if with_b1:
                        nc.vector.tensor_scalar(
                            hT[:, m, :],
                            p1[:],
                            b1_ts[s][:, m : m + 1],
                            0.0,
                            mybir.AluOpType.add,
                            mybir.AluOpType.max,
                        )
                    else:
                        nc.vector.tensor_scalar_max(hT[:, m, :], p1[:], 0.0)
                hTs.append(hT)
                y_ts.append(sb.tile([P, A], y_dt, tag="y", name=f"y_{s}"))
                p2s.append(
                    [
                        ps.tile([P, NF], f32, tag=f"p2_{n}", name=f"p2_{s}_{n}")
                        for n in range(NA)
                    ]
                )

            # Layer 2, in W2-chunk arrival order (slot-major).
            for s in range(SLOTS):
                for m in range(KH):
                    if with_b2 and m == 0:
                        for n in range(NA):
                            nc.tensor.matmul(
                                p2s[s][n][:],
                                e0[:],
                                b2_ts[s][:, ts(n, NF)],
                                start=True,
                                stop=False,
                            )
                    for n in range(NA):
                        nc.tensor.matmul(
                            p2s[s][n][:],
                            hTs[s][:, m, :],
                            b_ts[s][m][:, ts(n, NF)],
                            start=(m == 0 and not with_b2),
                            stop=(m == KH - 1),
                        )
                        if m == KH - 1:
                            nc.vector.tensor_copy(
                                y_ts[s][:, ts(n, NF)], p2s[s][n][:]
                            )
                            nc.sync.dma_start(
                                y_d[s][:, ts(n, NF)], y_ts[s][:, ts(n, NF)]
                            )

    nc.compile()
    return nc


def _get_nc(compute_dt, with_b1, with_b2):
    key = (compute_dt, with_b1, with_b2)
    if key not in _NC_CACHE:
        _NC_CACHE[key] = _build(*key)
    return _NC_CACHE[key]


def kernel(**inputs):
    global LAST_RESULTS
    mask = np.ascontiguousarray(np.asarray(inputs["mask"], dtype=np.float32))
    instance = np.asarray(inputs["instance"]).astype(np.int64)
    W1 = np.asarray(inputs["W1"], dtype=np.float32)
    b1 = np.asarray(inputs["b1"], dtype=np.float32)
    W2 = np.asarray(inputs["W2"], dtype=np.float32)
    b2 = np.asarray(inputs["b2"], dtype=np.float32)

    cdt, npdt = _dtypes(COMPUTE_DT)
    with_b1 = bool(np.any(b1))
    with_b2 = bool(np.any(b2))
    nc = _get_nc(COMPUTE_DT, with_b1, with_b2)

    x = mask.reshape(B, D)
    xp = np.zeros((B, DP), np.float32)
    xp[:, :D] = x
    xp = xp.astype(npdt, copy=False)

    # Weight layouts matching the SBUF tiles: partition dim first.
    W1p = np.zeros((E, DP, H), np.float32)
    W1p[:, :D, :] = W1
    w1_l = np.ascontiguousarray(
        W1p.reshape(E, KD, P, H).transpose(0, 2, 1, 3).reshape(E, P, KD * H)
    ).astype(npdt, copy=False)                            # [E, P, KD*H]
    w2_l = np.ascontiguousarray(
        W2.reshape(E, KH, P, A).transpose(0, 2, 1, 3).reshape(E, P, FB)
    ).astype(npdt, copy=False)                            # [E, P, FB]
    b1_l = np.ascontiguousarray(b1.reshape(E, KH, P).transpose(0, 2, 1))  # [E,P,KH]
    b2_l = b2.astype(npdt, copy=False)

    chunks = []
    for e in range(E):
        idx = np.nonzero(instance == e)[0]
        for i in range(0, len(idx), P):
            chunks.append((e, idx[i : i + P]))
    per_round = NCORES * SLOTS
    rounds = max(1, -(-len(chunks) // per_round))

    y = np.zeros((B, A), np.float32)
    for r in range(rounds):
        in_maps = []
        slot_idx = []  # (core, slot) -> sample indices
        for c in range(NCORES):
            ab = np.zeros((SLOTS, P, FA), npdt)
            bb = np.zeros((SLOTS, P, FB), npdt)
            b1a = np.zeros((SLOTS, P, KH), np.float32)
            b2a = np.zeros((SLOTS, A), npdt)
            cidx = []
            for s in range(SLOTS):
                k = r * per_round + c * SLOTS + s
                if k < len(chunks):
                    e, idx = chunks[k]
                    L = len(idx)
                    xg = xp[idx]  # [L, DP]
                    xt = ab[s, :, : KD * P].reshape(P, KD, P)
                    for o in range(KD):
                        xt[:, o, :L] = xg[:, o * P : (o + 1) * P].T
                    ab[s, :, KD * P :] = w1_l[e]
                    bb[s] = w2_l[e]
                    b1a[s] = b1_l[e]
                    b2a[s] = b2_l[e]
                    cidx.append(idx)
                else:
                    cidx.append(None)
            slot_idx.append(cidx)
            m = {"a": ab, "b": bb}
            if with_b1:
                m["b1"] = b1a
            if with_b2:
                m["b2"] = b2a
            in_maps.append(m)

        res = run_bass_kernel_spmd(nc, in_maps, core_ids=list(range(NCORES)))
        LAST_RESULTS = res
        for c in range(NCORES):
            yc = np.asarray(res.results[c]["y"], dtype=np.float32)
            for s in range(SLOTS):
                idx = slot_idx[c][s]
                if idx is not None:
                    y[idx] = yc[s, : len(idx)]

    return y
